# revision 1
# baseline (speedup 1.0000x reference)
"""BiDAF attention kernel for trn2 (8 NeuronCores, pure data parallel).

v2: instruction-count-optimized.  S^T layout (LP on partitions) so both
softmaxes reduce along the free axis.  Aq = w^T q^T precomputed for all 16
batches in batched N=400 matmuls; h computed with reversed operands
(lhsT = eq column, rhs = p rows) with the normalizing sum fused in via a
ones-column appended to p; output rows assembled in SBUF so each LP-chunk
is a single contiguous-row DMA.
"""

from contextlib import ExitStack

import numpy as np

import concourse.bass as bass
import concourse.mybir as mybir
import concourse.tile as tile
from concourse.bass_utils import run_bass_kernel_spmd
from concourse.masks import make_identity

F32 = mybir.dt.float32
AX = mybir.AxisListType
ALU = mybir.AluOpType
ACTF = mybir.ActivationFunctionType

B, LP, LQ, H = 128, 400, 100, 256
NCORES = 8
BP = B // NCORES  # batches per core
ROWS = [(0, 128), (128, 128), (256, 128), (384, 16)]
BIG = float(np.float32(3.0e38))


def build_nc():
    nc = bass.Bass("TRN2", target_bir_lowering=False, debug=False)

    pn = nc.dram_tensor("pn", [BP, LP, H], F32, kind="ExternalInput")
    pt = nc.dram_tensor("pt", [BP, H, LP], F32, kind="ExternalInput")
    qn = nc.dram_tensor("qn", [BP, LQ, H], F32, kind="ExternalInput")
    qt = nc.dram_tensor("qt", [BP, H, LQ], F32, kind="ExternalInput")
    w = nc.dram_tensor("w", [H, H], F32, kind="ExternalInput")
    g = nc.dram_tensor("g", [BP, LP, 4 * H], F32, kind="ExternalOutput")

    with tile.TileContext(nc) as tc, ExitStack() as ctx:
        cpool = ctx.enter_context(tc.tile_pool(name="consts", bufs=1))
        wp = ctx.enter_context(tc.tile_pool(name="work", bufs=4))
        ppb = ctx.enter_context(tc.tile_pool(name="psb", bufs=2, space="PSUM"))
        pps = ctx.enter_context(tc.tile_pool(name="pss", bufs=2, space="PSUM"))
        ppu = ctx.enter_context(tc.tile_pool(name="psu", bufs=2, space="PSUM"))
        ppt = ctx.enter_context(tc.tile_pool(name="pst", bufs=2, space="PSUM"))

        # ---- constants ----
        ident = cpool.tile([128, 128], F32)
        make_identity(nc, ident[:])
        ones_r = cpool.tile([1, 128], F32)
        nc.vector.memset(ones_r[:], 1.0)
        Wt = cpool.tile([128, 2, H], F32)
        nc.sync.dma_start(Wt[:, 0, :], w[0:128, :])
        nc.sync.dma_start(Wt[:, 1, :], w[128:256, :])

        # ---- pre-pass: Aq = w^T q^T for ALL batches, batched 4-wide ----
        QtA = cpool.tile([128, 2, BP * LQ], F32)       # (h-part, kc, b*l)
        AqA = cpool.tile([128, 2, BP * LQ], F32)
        for gi in range(BP // 4):
            for kc in range(2):
                nc.sync.dma_start(
                    QtA[:, kc, gi * 400:(gi + 1) * 400].rearrange(
                        "p (b l) -> p b l", b=4),
                    qt[gi * 4:(gi + 1) * 4,
                       kc * 128:(kc + 1) * 128, :].rearrange(
                        "b p l -> p b l"))
        for gi in range(BP // 4):
            for ms in range(2):
                psAq = ppb.tile([128, 400], F32, tag="big")
                for kc in range(2):
                    nc.tensor.matmul(
                        psAq[:],
                        Wt[:, kc, ms * 128:(ms + 1) * 128],
                        QtA[:, kc, gi * 400:(gi + 1) * 400],
                        start=(kc == 0), stop=(kc == 1),
                    )
                nc.scalar.copy(AqA[:, ms, gi * 400:(gi + 1) * 400], psAq[:])

        for gi in range(BP // 4):
            grp = []
            NMN4 = wp.tile([128, 4], F32, tag="NMN4")
            for j in range(4):
                b = gi * 4 + j
                bq = b * LQ
                # ---------------- loads ----------------
                Pn = wp.tile([128, 4, H + 1], F32, tag="Pn", bufs=5)  # col 256: ones
                nc.vector.memset(Pn[:, :, H:H + 1], 1.0)
                nc.sync.dma_start(
                    Pn[0:128, 0:3, 0:H],
                    pn[b, 0:384, :].rearrange("(i r) h -> r i h", r=128))
                nc.sync.dma_start(Pn[0:16, 3, 0:H], pn[b, 384:400, :])
                Pt = wp.tile([128, 2, LP], F32, tag="Pt")
                nc.sync.dma_start(Pt[:],
                                  pt[b].rearrange("(k p) l -> p k l", p=128))
                Qn = wp.tile([128, H], F32, tag="Qn")
                nc.sync.dma_start(Qn[0:LQ, :], qn[b])

                # ---------------- S^T = p @ Aq  (LP, LQ) ----------------
                psS01 = pps.tile([128, 2, LQ], F32, tag="st")
                psS23 = pps.tile([128, 2, LQ], F32, tag="st")
                psSc = [psS01, psS01, psS23, psS23]
                for i, (off, r) in enumerate(ROWS):
                    for kc in range(2):
                        nc.tensor.matmul(
                            psSc[i][0:r, i % 2, :],
                            Pt[:, kc, off:off + r],
                            AqA[:, kc, bq:bq + LQ],
                            start=(kc == 0), stop=(kc == 1),
                        )

                # ---------------- C2Q softmax over LQ ----------------
                NM = wp.tile([128, 4], F32, tag="NM", bufs=5)
                RS = wp.tile([128, 4], F32, tag="RS")
                nc.vector.memset(NM[:], BIG)
                nc.vector.memset(RS[:], 1.0)
                for i, (off, r) in enumerate(ROWS):
                    nc.vector.tensor_reduce(
                        NM[0:r, i:i + 1], psSc[i][0:r, i % 2, :],
                        axis=AX.X, op=ALU.max, negate=True,
                    )
                E = wp.tile([128, 4, LQ], F32, tag="E")
                for i, (off, r) in enumerate(ROWS):
                    nc.scalar.activation(
                        E[0:r, i, :], psSc[i][0:r, i % 2, :], ACTF.Exp,
                        bias=NM[0:r, i:i + 1], accum_out=RS[0:r, i:i + 1],
                    )
                RCP = wp.tile([128, 4], F32, tag="RCP")
                nc.vector.reciprocal(RCP[:], RS[:])
                AT = wp.tile([128, 4, LQ], F32, tag="AT")
                for i, (off, r) in enumerate(ROWS):
                    nc.vector.tensor_scalar_mul(
                        AT[0:r, i, :], E[0:r, i, :], RCP[0:r, i:i + 1])

                # ---------------- transpose a^T -> a ----------------
                psAm = ppb.tile([128, 4, 128], F32, tag="big")
                for i, (off, r) in enumerate(ROWS):
                    nc.tensor.transpose(
                        psAm[0:LQ, i, 0:r], AT[0:r, i, :], ident[0:r, 0:r])
                Am = wp.tile([128, 4, 128], F32, tag="Am")
                nc.scalar.copy(Am[0:LQ, 0:3, :], psAm[0:LQ, 0:3, :])
                nc.scalar.copy(Am[0:LQ, 3, 0:16], psAm[0:LQ, 3, 0:16])

                # ---------------- U^T + output assembly ----------------
                Gt = wp.tile([128, 4, 4 * H], F32, tag="Gt", bufs=5)
                nc.vector.tensor_copy(Gt[0:128, 0:3, 0:H],
                                      Pn[0:128, 0:3, 0:H])
                nc.vector.tensor_copy(Gt[0:16, 3, 0:H], Pn[0:16, 3, 0:H])
                for i, (off, r) in enumerate(ROWS):
                    psU = ppu.tile([128, H], F32, tag="u")
                    nc.tensor.matmul(
                        psU[0:r, :], Am[0:LQ, i, 0:r], Qn[0:LQ, :],
                        start=True, stop=True,
                    )
                    nc.vector.tensor_tensor(
                        Gt[0:r, i, 2 * H:3 * H], Pn[0:r, i, 0:H],
                        psU[0:r, :], op=ALU.mult)

                nc.vector.tensor_reduce(NMN4[:, j:j + 1], NM[:], axis=AX.X,
                                        op=ALU.min)
                grp.append((b, Pn, Gt, NM))

            # ---------------- group Q2C glue (all 4 batches) ----------------
            psTB = ppt.tile([128, 257], F32, tag="tiny")
            nc.tensor.transpose(psTB[0:4, 0:128], NMN4[:], ident[:])
            ngmin4 = wp.tile([4, 1], F32, tag="ngmin4")   # -gmax per batch
            nc.vector.tensor_reduce(ngmin4[:], psTB[0:4, 0:128], axis=AX.X,
                                    op=ALU.min)
            nc.tensor.transpose(psTB[0:1, 132:136], ngmin4[:],
                                ident[0:4, 0:4])
            ngr = wp.tile([1, 4], F32, tag="ngr")
            nc.scalar.copy(ngr[:], psTB[0:1, 132:136])
            psB4 = ppt.tile([128, 257], F32, tag="tiny")
            nc.tensor.matmul(psB4[0:128, 0:4], ones_r[:], ngr[:],
                             start=True, stop=True)
            nb4 = wp.tile([128, 4], F32, tag="nb4")
            nc.scalar.copy(nb4[:], psB4[0:128, 0:4])

            for j in range(4):
                b, Pn, Gt, NM = grp[j]
                EQ = wp.tile([128, 4], F32, tag="EQ")
                nc.scalar.activation(EQ[:], NM[:], ACTF.Exp,
                                     bias=nb4[:, j:j + 1], scale=-1.0)
                psHr = ppt.tile([128, 257], F32, tag="tiny")
                for i, (off, r) in enumerate(ROWS):
                    nc.tensor.matmul(
                        psHr[0:1, :],
                        EQ[0:r, i:i + 1],
                        Pn[0:r, i, 0:H + 1],
                        start=(i == 0), stop=(i == 3),
                    )
                rq = wp.tile([1, 1], F32, tag="rq")
                nc.vector.reciprocal(rq[:], psHr[0:1, H:H + 1])
                hrow = wp.tile([1, H], F32, tag="hrow")
                nc.scalar.mul(hrow[:], psHr[0:1, 0:H], rq[:])
                psHt = ppb.tile([128, H], F32, tag="big")
                nc.tensor.matmul(psHt[:], ones_r[:], hrow[:], start=True,
                                 stop=True)
                HtS = wp.tile([128, H], F32, tag="HtS")
                nc.vector.tensor_copy(HtS[:], psHt[:])
                for i, (off, r) in enumerate(ROWS):
                    nc.vector.tensor_copy(Gt[0:r, i, H:2 * H], HtS[0:r, :])
                    nc.gpsimd.tensor_tensor(
                        Gt[0:r, i, 3 * H:4 * H], Pn[0:r, i, 0:H],
                        HtS[0:r, :], op=ALU.mult)
                for i, (off, r) in enumerate(ROWS):
                    nc.sync.dma_start(g[b, off:off + r, :], Gt[0:r, i, :])

    return nc


def legalize_waits(nc):
    """Split multi-wait instructions into single-wait NoOps + instruction.

    The TPB ISA has exactly one (wait, update) EVENTS slot per 64B
    instruction; this walrus build refuses instructions with more than one
    sync wait ("Too many sync wait commands").  Tile's scheduler emits
    vector-clock waits freely, so legalize here: excess waits move onto
    engine-queue NoOps placed immediately before the instruction.
    """
    counter = 0
    for f in nc.m.functions:
        for blk in f.blocks:
            new = []
            for inst in blk.instructions:
                si = getattr(inst, "sync_info", None)
                if si is not None and len(si.on_wait) > 1:
                    waits = list(si.on_wait)
                    assert len(si.on_update) <= 1, inst
                    for wt in waits[:-1]:
                        counter += 1
                        new.append(mybir.InstNoOp(
                            name=f"I-waitnop-{counter}",
                            engine=inst.engine,
                            sync_info=mybir.SyncInfo(on_wait=[wt],
                                                     on_update=[]),
                        ))
                    inst.sync_info = mybir.SyncInfo(
                        on_wait=[waits[-1]], on_update=list(si.on_update))
                new.append(inst)
            blk.instructions = new
    return nc


def _make_in_maps(p, q, w):
    p = np.ascontiguousarray(p, dtype=np.float32)
    q = np.ascontiguousarray(q, dtype=np.float32)
    w = np.ascontiguousarray(w, dtype=np.float32)
    in_maps = []
    for c in range(NCORES):
        sl = slice(c * BP, (c + 1) * BP)
        in_maps.append({
            "pn": p[sl],
            "pt": np.ascontiguousarray(p[sl].transpose(0, 2, 1)),
            "qn": q[sl],
            "qt": np.ascontiguousarray(q[sl].transpose(0, 2, 1)),
            "w": w,
        })
    return in_maps


def run(p, q, w, trace=False):
    nc = legalize_waits(build_nc())
    res = run_bass_kernel_spmd(
        nc, _make_in_maps(p, q, w), list(range(NCORES)), trace=trace)
    out = np.concatenate([res.results[c]["g"] for c in range(NCORES)], axis=0)
    return out, res


def kernel(p, q, w):
    out, _ = run(p, q, w, trace=False)
    return out



# revision 6
# speedup vs baseline: 1.3850x; 1.3850x over previous
"""BiDAF attention kernel for trn2 (8 NeuronCores, pure data parallel).

v3: mixed-precision + DMA-minimized.

- S = q W p chain stays fp32 (softmax logits have std ~256; bf16's ~1.5
  absolute logit error flips argmaxes and breaks both softmaxes).  Every
  other tensor is bf16: inputs pn/qn ship as bf16, all post-softmax
  matmuls (transpose-a, U, h, Ht) run at the 1 cyc/row bf16 PE rate, and
  the output g is written bf16 (host upcasts) — halving output HBM
  traffic.
- Uniform 100-row LP chunks (4 per batch) instead of 3x128+16.
- DMAs batched per 4-batch group (23 DMA instructions total): inputs
  group-loaded, P / Ht / [PU|PHt] output blocks each written by one DMA
  per group.  P block streams straight from the Pn input tile; Ht block
  streams from a (100,4,1,256) tile with a stride-0 broadcast AP.
- PSUM-exit + elementwise work spread across ACT (exp, copies), DVE
  (reduces, fused (U*rcp)*P via scalar_tensor_tensor) and GPSIMD (P*Ht).
"""

from contextlib import ExitStack

import numpy as np
import ml_dtypes

import concourse.bass as bass
import concourse.mybir as mybir
import concourse.tile as tile
from concourse.bass_utils import run_bass_kernel_spmd
from concourse.masks import make_identity

F32 = mybir.dt.float32
BF16 = mybir.dt.bfloat16
AX = mybir.AxisListType
ALU = mybir.AluOpType
ACTF = mybir.ActivationFunctionType

B, LP, LQ, H = 128, 400, 100, 256
NCORES = 8
BP = B // NCORES   # batches per core
NG = BP // 4       # 4-batch groups per core
R = 100            # LP chunk rows (4 uniform chunks)


def build_nc():
    nc = bass.Bass("TRN2", target_bir_lowering=False, debug=False)

    pn = nc.dram_tensor("pn", [BP, LP, H], BF16, kind="ExternalInput")
    pt = nc.dram_tensor("pt", [BP, H, LP], F32, kind="ExternalInput")
    qta = nc.dram_tensor("qta", [H, BP * LQ], F32, kind="ExternalInput")
    qn = nc.dram_tensor("qn", [BP, LQ, H], BF16, kind="ExternalInput")
    w = nc.dram_tensor("w", [H, H], F32, kind="ExternalInput")
    g = nc.dram_tensor("g", [BP, LP, 4 * H], BF16, kind="ExternalOutput")

    with tile.TileContext(nc) as tc, ExitStack() as ctx:
        cpool = ctx.enter_context(tc.tile_pool(name="consts", bufs=1))
        wp = ctx.enter_context(tc.tile_pool(name="work", bufs=2))
        pst = ctx.enter_context(tc.tile_pool(name="pst", bufs=2, space="PSUM"))
        pam = ctx.enter_context(tc.tile_pool(name="pam", bufs=2, space="PSUM"))
        ppu = ctx.enter_context(tc.tile_pool(name="ppu", bufs=2, space="PSUM"))
        ppt = ctx.enter_context(tc.tile_pool(name="ppt", bufs=2, space="PSUM"))

        # ---- constants ----
        identF = cpool.tile([128, 128], F32)
        make_identity(nc, identF[:])
        identB = cpool.tile([128, 128], BF16)
        make_identity(nc, identB[:])
        ones_rF = cpool.tile([1, 128], F32)
        nc.vector.memset(ones_rF[:], 1.0)
        ones_rB = cpool.tile([1, 128], BF16)
        nc.vector.memset(ones_rB[:], 1.0)

        Wt = cpool.tile([128, 2, H], F32)
        nc.sync.dma_start(Wt[:], w[:].rearrange("(kc p) d -> p kc d", p=128))
        QtA = cpool.tile([128, 2, BP * LQ], F32)
        nc.sync.dma_start(QtA[:],
                          qta[:].rearrange("(kc p) x -> p kc x", p=128))
        QnA = cpool.tile([LQ, BP, H], BF16)
        nc.sync.dma_start(QnA[:], qn[:].rearrange("b l h -> l b h"))

        # ---- prepass: Aq[d, (b l)] = sum_h w[h, d] q[b, l, h]  (fp32) ----
        AqA = cpool.tile([128, 2, BP * LQ], F32)
        for gi in range(NG):
            for ms in range(2):
                psAq = pst.tile([128, 400], F32, tag="st")
                for kc in range(2):
                    nc.tensor.matmul(
                        psAq[:],
                        Wt[:, kc, ms * 128:(ms + 1) * 128],
                        QtA[:, kc, gi * 400:(gi + 1) * 400],
                        start=(kc == 0), stop=(kc == 1),
                    )
                nc.scalar.copy(AqA[:, ms, gi * 400:(gi + 1) * 400], psAq[:])

        for gi in range(NG):
            b0 = gi * 4
            # ---------------- group loads ----------------
            PtG = wp.tile([128, 4, 2, LP], F32, tag="PtG")
            nc.sync.dma_start(
                PtG[:],
                pt[b0:b0 + 4].rearrange("b (kc d) l -> d b kc l", d=128))
            PnG = wp.tile([R, 4, 4, H + 1], BF16, tag="PnG", bufs=3)
            nc.vector.memset(PnG[:, :, :, H:H + 1], 1.0)
            nc.sync.dma_start(
                PnG[:, :, :, 0:H],
                pn[b0:b0 + 4].rearrange("b (i r) h -> r b i h", r=R))

            GtG = wp.tile([R, 4, 4, 2 * H], BF16, tag="GtG")
            HtG = wp.tile([R, 4, 1, H], BF16, tag="HtG")

            grp = []
            NMN4 = wp.tile([R, 4], F32, tag="NMN4")
            # ---------------- phase A: C2Q per batch ----------------
            for j in range(4):
                b = b0 + j
                bq = b * LQ
                # S^T = p @ Aq  (chunked LP x LQ), fp32
                psSt = pst.tile([R, 4, LQ], F32, tag="st")
                for i in range(4):
                    for kc in range(2):
                        nc.tensor.matmul(
                            psSt[:, i, :],
                            PtG[:, j, kc, i * R:(i + 1) * R],
                            AqA[:, kc, bq:bq + LQ],
                            start=(kc == 0), stop=(kc == 1),
                        )
                NM = wp.tile([R, 4], F32, tag="NM", bufs=6)
                nc.vector.tensor_reduce(NM[:], psSt[:], axis=AX.X,
                                        op=ALU.max, negate=True)
                E = wp.tile([R, 4, LQ], BF16, tag="E")
                RS = wp.tile([R, 4], F32, tag="RS")
                for i in range(4):
                    nc.scalar.activation(
                        E[:, i, :], psSt[:, i, :], ACTF.Exp,
                        bias=NM[:, i:i + 1], accum_out=RS[:, i:i + 1],
                    )
                RCP = wp.tile([R, 4], F32, tag="RCP")
                nc.vector.reciprocal(RCP[:], RS[:])

                # a = E^T (unnormalized); U^T = a_chunk @ Qn
                psAm = pam.tile([R, 4, LQ], BF16, tag="am")
                for i in range(4):
                    nc.tensor.transpose(
                        psAm[0:LQ, i, 0:R], E[:, i, :], identB[0:R, 0:R])
                Am = wp.tile([LQ, 4, R], BF16, tag="Am")
                nc.scalar.copy(Am[:], psAm[0:LQ, :, :])
                # two psU half-tiles, 2 chunks each
                for half in range(2):
                    psU = ppu.tile([R, 2, H], F32, tag="u")
                    for ih in range(2):
                        i = half * 2 + ih
                        nc.tensor.matmul(
                            psU[:, ih, :], Am[:, i, :], QnA[:, b, :],
                            start=True, stop=True,
                        )
                    for ih in range(2):
                        i = half * 2 + ih
                        nc.vector.scalar_tensor_tensor(
                            GtG[:, j, i, 0:H],
                            psU[:, ih, :], RCP[:, i:i + 1],
                            PnG[:, j, i, 0:H],
                            op0=ALU.mult, op1=ALU.mult,
                        )
                nc.vector.tensor_reduce(NMN4[:, j:j + 1], NM[:], axis=AX.X,
                                        op=ALU.min)
                grp.append((b, NM))

            # ---------------- group Q2C glue ----------------
            psTB = ppt.tile([128, 257], F32, tag="tiny")
            nc.tensor.transpose(psTB[0:4, 0:R], NMN4[:], identF[0:R, 0:R])
            ngmin4 = wp.tile([4, 1], F32, tag="ngmin4")   # -gmax per batch
            nc.vector.tensor_reduce(ngmin4[:], psTB[0:4, 0:R], axis=AX.X,
                                    op=ALU.min)
            psT2 = ppt.tile([128, 257], F32, tag="tiny")
            nc.tensor.transpose(psT2[0:1, 0:4], ngmin4[:], identF[0:4, 0:4])
            ngr = wp.tile([1, 4], F32, tag="ngr")
            nc.vector.tensor_copy(ngr[:], psT2[0:1, 0:4])
            psB4 = ppt.tile([128, 257], F32, tag="tiny")
            nc.tensor.matmul(psB4[0:R, 0:4], ones_rF[0:1, 0:R], ngr[:],
                             start=True, stop=True)
            nb4 = wp.tile([R, 4], F32, tag="nb4")
            nc.vector.tensor_copy(nb4[:], psB4[0:R, 0:4])

            # ---------------- phase B: Q2C per batch ----------------
            for j in range(4):
                b, NM = grp[j]
                EQ = wp.tile([R, 4], BF16, tag="EQ")
                nc.scalar.activation(EQ[:], NM[:], ACTF.Exp,
                                     bias=nb4[:, j:j + 1], scale=-1.0)
                psHr = ppt.tile([128, 257], F32, tag="tiny")
                for i in range(4):
                    nc.tensor.matmul(
                        psHr[0:1, 0:H + 1],
                        EQ[:, i:i + 1],
                        PnG[:, j, i, :],
                        start=(i == 0), stop=(i == 3),
                    )
                rq = wp.tile([1, 1], F32, tag="rq")
                nc.vector.reciprocal(rq[:], psHr[0:1, H:H + 1])
                hrow = wp.tile([1, H], BF16, tag="hrow")
                nc.scalar.mul(hrow[:], psHr[0:1, 0:H], rq[:])
                psHt = ppt.tile([128, 257], F32, tag="tiny")
                nc.tensor.matmul(psHt[0:R, 0:H], ones_rB[0:1, 0:R], hrow[:],
                                 start=True, stop=True)
                nc.scalar.copy(HtG[:, j, 0, :], psHt[0:R, 0:H])
                nc.gpsimd.tensor_tensor(
                    GtG[:, j, :, H:2 * H], PnG[:, j, :, 0:H],
                    HtG[:, j, :, :].broadcast_to((R, 4, H)), op=ALU.mult)

            # ---------------- group stores ----------------
            nc.sync.dma_start(
                g[b0:b0 + 4, :, 0:H].rearrange("b (i r) h -> r b i h", r=R),
                PnG[:, :, :, 0:H])
            for i in range(4):
                nc.scalar.dma_start(
                    g[b0:b0 + 4, i * R:(i + 1) * R, H:2 * H].rearrange(
                        "b r h -> r b h"),
                    HtG[:, :, 0, :])
            nc.sync.dma_start(
                g[b0:b0 + 4, :, 2 * H:4 * H].rearrange(
                    "b (i r) h -> r b i h", r=R),
                GtG[:])

    return nc


def legalize_waits(nc):
    """Split multi-wait instructions into single-wait NoOps + instruction.

    The TPB ISA has exactly one (wait, update) EVENTS slot per 64B
    instruction; this walrus build refuses instructions with more than one
    sync wait ("Too many sync wait commands").  Tile's scheduler emits
    vector-clock waits freely, so legalize here: excess waits move onto
    engine-queue NoOps placed immediately before the instruction.
    """
    counter = 0
    for f in nc.m.functions:
        for blk in f.blocks:
            new = []
            for inst in blk.instructions:
                si = getattr(inst, "sync_info", None)
                if si is not None and len(si.on_wait) > 1:
                    waits = list(si.on_wait)
                    assert len(si.on_update) <= 1, inst
                    for wt in waits[:-1]:
                        counter += 1
                        new.append(mybir.InstNoOp(
                            name=f"I-waitnop-{counter}",
                            engine=inst.engine,
                            sync_info=mybir.SyncInfo(on_wait=[wt],
                                                     on_update=[]),
                        ))
                    inst.sync_info = mybir.SyncInfo(
                        on_wait=[waits[-1]], on_update=list(si.on_update))
                new.append(inst)
            blk.instructions = new
    return nc


def _make_in_maps(p, q, w):
    p = np.ascontiguousarray(p, dtype=np.float32)
    q = np.ascontiguousarray(q, dtype=np.float32)
    w = np.ascontiguousarray(w, dtype=np.float32)
    bf = ml_dtypes.bfloat16
    in_maps = []
    for c in range(NCORES):
        sl = slice(c * BP, (c + 1) * BP)
        ps = p[sl]
        qs = q[sl]
        in_maps.append({
            "pn": ps.astype(bf),
            "pt": np.ascontiguousarray(ps.transpose(0, 2, 1)),
            "qta": np.ascontiguousarray(
                qs.transpose(2, 0, 1).reshape(H, BP * LQ)),
            "qn": qs.astype(bf),
            "w": w,
        })
    return in_maps


def run(p, q, w, trace=False):
    nc = legalize_waits(build_nc())
    res = run_bass_kernel_spmd(
        nc, _make_in_maps(p, q, w), list(range(NCORES)), trace=trace)
    out = np.concatenate(
        [res.results[c]["g"].astype(np.float32) for c in range(NCORES)],
        axis=0)
    return out, res


def kernel(p, q, w):
    out, _ = run(p, q, w, trace=False)
    return out


# revision 8
# speedup vs baseline: 1.5478x; 1.1176x over previous
"""BiDAF attention kernel for trn2 (8 NeuronCores, pure data parallel).

v4: mixed-precision, descriptor-minimized DMA.

- S = q W p chain stays fp32 (softmax logits have std ~256; bf16's ~1.5
  absolute logit error flips argmaxes and breaks both softmaxes).  Every
  other tensor is bf16.
- All DRAM tensors are host-side pre-permuted to exactly match the SBUF
  tile layouts, so each DMA moves per-partition-contiguous 8-16KB runs:
  ~100 descriptors per transfer instead of ~1600x512B.  (HWDGE
  descriptor generation costs ~2ns/descriptor of sequencer time and
  small descriptors also cap HBM throughput.)
- The C2Q softmax denominator rides the U matmul: per chunk, with the
  a-chunk weights already loaded, a second N=1 matmul against a ones
  column produces RS per LP row in a tiny PSUM tile (no activation
  accumulators).  Normalization is fused into the P*U elementwise op via
  scalar_tensor_tensor: (psU * rcp) * Pn.
- Ht rows are broadcast across partitions by a small SBUF->SBUF
  stride-0 DMA; the DRAM Ht block is written once (1, 4, 256) per group
  and the host tiles it to (400, 256) during unshard.
"""

from contextlib import ExitStack

import numpy as np
import ml_dtypes

import concourse.bass as bass
import concourse.mybir as mybir
import concourse.tile as tile
from concourse.bass_utils import run_bass_kernel_spmd
from concourse.masks import make_identity

F32 = mybir.dt.float32
BF16 = mybir.dt.bfloat16
AX = mybir.AxisListType
ALU = mybir.AluOpType
ACTF = mybir.ActivationFunctionType

B, LP, LQ, H = 128, 400, 100, 256
NCORES = 8
BP = B // NCORES   # batches per core
NG = BP // 4       # 4-batch groups per core
R = 100            # LP chunk rows (4 uniform chunks)


def build_nc():
    nc = bass.Bass("TRN2", target_bir_lowering=False, debug=False)

    # host-permuted layouts (match SBUF tiles exactly)
    pnp = nc.dram_tensor("pnp", [NG, R, 4, 4, H], BF16, kind="ExternalInput")
    ptp = nc.dram_tensor("ptp", [NG, 128, 4, 2, LP], F32,
                         kind="ExternalInput")
    qtp = nc.dram_tensor("qtp", [128, 2, BP * LQ], F32, kind="ExternalInput")
    qnp = nc.dram_tensor("qnp", [LQ, BP, H], BF16, kind="ExternalInput")
    wtp = nc.dram_tensor("wtp", [128, 2, H], F32, kind="ExternalInput")
    gP = nc.dram_tensor("gP", [NG, R, 4, 4, H], BF16, kind="ExternalOutput")
    gHt = nc.dram_tensor("gHt", [NG, 4, H], BF16, kind="ExternalOutput")
    gG2 = nc.dram_tensor("gG2", [NG, R, 4, 4, 2 * H], BF16,
                         kind="ExternalOutput")

    with tile.TileContext(nc) as tc, ExitStack() as ctx:
        cpool = ctx.enter_context(tc.tile_pool(name="consts", bufs=1))
        wp = ctx.enter_context(tc.tile_pool(name="work", bufs=2))
        pst = ctx.enter_context(tc.tile_pool(name="pst", bufs=2, space="PSUM"))
        pam = ctx.enter_context(tc.tile_pool(name="pam", bufs=2, space="PSUM"))
        ppu = ctx.enter_context(tc.tile_pool(name="ppu", bufs=2, space="PSUM"))
        ppt = ctx.enter_context(tc.tile_pool(name="ppt", bufs=2, space="PSUM"))

        # ---- constants ----
        identF = cpool.tile([128, 128], F32)
        make_identity(nc, identF[:])
        identB = cpool.tile([128, 128], BF16)
        make_identity(nc, identB[:])
        ones_rF = cpool.tile([1, 128], F32)
        nc.vector.memset(ones_rF[:], 1.0)
        onesCb = cpool.tile([128, 1], BF16)
        nc.vector.memset(onesCb[:], 1.0)
        ones_rB = cpool.tile([1, 128], BF16)
        nc.vector.memset(ones_rB[:], 1.0)

        Wt = cpool.tile([128, 2, H], F32)
        nc.sync.dma_start(Wt[:], wtp[:])
        QtA = cpool.tile([128, 2, BP * LQ], F32)
        nc.sync.dma_start(QtA[:], qtp[:])
        QnA = cpool.tile([LQ, BP, H], BF16)
        nc.scalar.dma_start(QnA[:], qnp[:])

        # ---- prepass: Aq[d, (b l)] = sum_h w[h, d] q[b, l, h]  (fp32) ----
        AqA = cpool.tile([128, 2, BP * LQ], F32)
        for gi in range(NG):
            for ms in range(2):
                psAq = pst.tile([128, 400], F32, tag="st")
                for kc in range(2):
                    nc.tensor.matmul(
                        psAq[:],
                        Wt[:, kc, ms * 128:(ms + 1) * 128],
                        QtA[:, kc, gi * 400:(gi + 1) * 400],
                        start=(kc == 0), stop=(kc == 1),
                    )
                nc.scalar.copy(AqA[:, ms, gi * 400:(gi + 1) * 400], psAq[:])

        for gi in range(NG):
            b0 = gi * 4
            # ---------------- group loads ----------------
            PtG = wp.tile([128, 4, 2, LP], F32, tag="PtG")
            nc.sync.dma_start(PtG[:], ptp[gi])
            PnG = wp.tile([R, 4, 4, H], BF16, tag="PnG", bufs=3)
            nc.sync.dma_start(PnG[:], pnp[gi])

            GtG = wp.tile([R, 4, 4, 2 * H], BF16, tag="GtG")
            HtG = wp.tile([R, 4, H], BF16, tag="HtG")

            grp = []
            NMN4 = wp.tile([R, 4], F32, tag="NMN4")
            # ---------------- phase A: C2Q per batch ----------------
            for j in range(4):
                b = b0 + j
                bq = b * LQ
                # S^T = p @ Aq  (chunked LP x LQ), fp32
                psSt = pst.tile([R, 4, LQ], F32, tag="st")
                for i in range(4):
                    for kc in range(2):
                        nc.tensor.matmul(
                            psSt[:, i, :],
                            PtG[:, j, kc, i * R:(i + 1) * R],
                            AqA[:, kc, bq:bq + LQ],
                            start=(kc == 0), stop=(kc == 1),
                        )
                NM = wp.tile([R, 4], F32, tag="NM", bufs=6)
                nc.vector.tensor_reduce(NM[:], psSt[:], axis=AX.X,
                                        op=ALU.max, negate=True)
                E = wp.tile([R, 4, LQ], BF16, tag="E")
                for i in range(4):
                    nc.scalar.activation(
                        E[:, i, :], psSt[:, i, :], ACTF.Exp,
                        bias=NM[:, i:i + 1],
                    )

                # a = E^T (unnormalized); U^T = a_chunk @ Qn, RS rides along
                psAm = pam.tile([R, 4, LQ], BF16, tag="am")
                for i in range(4):
                    nc.tensor.transpose(
                        psAm[0:LQ, i, 0:R], E[:, i, :], identB[0:R, 0:R])
                Am = wp.tile([LQ, 4, R], BF16, tag="Am")
                nc.scalar.copy(Am[:], psAm[0:LQ, :, :])
                psRS = ppt.tile([128, 260], F32, tag="tiny")
                psUs = []
                for half in range(2):
                    psU = ppu.tile([R, 2, H], F32, tag="u")
                    psUs.append(psU)
                    for ih in range(2):
                        i = half * 2 + ih
                        nc.tensor.matmul(
                            psU[:, ih, :], Am[:, i, :], QnA[:, b, :],
                            start=True, stop=True,
                        )
                        nc.tensor.matmul(
                            psRS[0:R, i:i + 1], Am[:, i, :], onesCb[0:LQ, :],
                            start=True, stop=True,
                        )
                RCP = wp.tile([R, 4], F32, tag="RCP")
                nc.vector.reciprocal(RCP[:], psRS[0:R, 0:4])
                for half in range(2):
                    for ih in range(2):
                        i = half * 2 + ih
                        nc.vector.scalar_tensor_tensor(
                            GtG[:, j, i, 0:H],
                            psUs[half][:, ih, :], RCP[:, i:i + 1],
                            PnG[:, j, i, :],
                            op0=ALU.mult, op1=ALU.mult,
                        )
                nc.vector.tensor_reduce(NMN4[:, j:j + 1], NM[:], axis=AX.X,
                                        op=ALU.min)
                grp.append((b, NM))

            # ---------------- group Q2C glue ----------------
            psTB = ppt.tile([128, 260], F32, tag="tiny")
            nc.tensor.transpose(psTB[0:4, 0:R], NMN4[:], identF[0:R, 0:R])
            ngmin4 = wp.tile([4, 1], F32, tag="ngmin4")   # -gmax per batch
            nc.vector.tensor_reduce(ngmin4[:], psTB[0:4, 0:R], axis=AX.X,
                                    op=ALU.min)
            psT2 = ppt.tile([128, 260], F32, tag="tiny")
            nc.tensor.transpose(psT2[0:1, 0:4], ngmin4[:], identF[0:4, 0:4])
            ngr = wp.tile([1, 4], F32, tag="ngr")
            nc.vector.tensor_copy(ngr[:], psT2[0:1, 0:4])
            psB4 = ppt.tile([128, 260], F32, tag="tiny")
            nc.tensor.matmul(psB4[0:R, 0:4], ones_rF[0:1, 0:R], ngr[:],
                             start=True, stop=True)
            nb4 = wp.tile([R, 4], F32, tag="nb4")
            nc.vector.tensor_copy(nb4[:], psB4[0:R, 0:4])

            # ---------------- phase B: Q2C per batch ----------------
            for j in range(4):
                b, NM = grp[j]
                EQ = wp.tile([R, 4], BF16, tag="EQ")
                nc.scalar.activation(EQ[:], NM[:], ACTF.Exp,
                                     bias=nb4[:, j:j + 1], scale=-1.0)
                psHr = ppt.tile([128, 260], F32, tag="tiny")
                for i in range(4):
                    nc.tensor.matmul(
                        psHr[0:1, 0:H],
                        EQ[:, i:i + 1],
                        PnG[:, j, i, :],
                        start=(i == 0), stop=(i == 3),
                    )
                nc.tensor.matmul(psHr[0:1, H:H + 4], onesCb[0:R, :], EQ[:],
                                 start=True, stop=True)
                smv = wp.tile([1, 1], F32, tag="smv")
                nc.vector.tensor_reduce(smv[:], psHr[0:1, H:H + 4],
                                        axis=AX.X, op=ALU.add)
                rq = wp.tile([1, 1], F32, tag="rq")
                nc.vector.reciprocal(rq[:], smv[:])
                hrow = wp.tile([1, H], BF16, tag="hrow")
                nc.scalar.mul(hrow[:], psHr[0:1, 0:H], rq[:])
                # broadcast h row across partitions via ones x hrow matmul
                psHt = ppt.tile([128, 260], F32, tag="tiny")
                nc.tensor.matmul(psHt[0:R, 0:H], ones_rB[0:1, 0:R],
                                 hrow[:], start=True, stop=True)
                nc.scalar.copy(HtG[:, j, :], psHt[0:R, 0:H])
                nc.gpsimd.tensor_tensor(
                    GtG[:, j, :, H:2 * H], PnG[:, j, :, :],
                    HtG[:, j:j + 1, :].broadcast_to((R, 4, H)), op=ALU.mult)

            # ---------------- group stores ----------------
            nc.scalar.dma_start(gP[gi], PnG[:])
            nc.scalar.dma_start(gHt[gi], HtG[0:1, :, :])
            nc.sync.dma_start(gG2[gi], GtG[:])

    return nc


def legalize_waits(nc):
    """Split multi-wait instructions into single-wait NoOps + instruction.

    The TPB ISA has exactly one (wait, update) EVENTS slot per 64B
    instruction; this walrus build refuses instructions with more than one
    sync wait ("Too many sync wait commands").  Tile's scheduler emits
    vector-clock waits freely, so legalize here: excess waits move onto
    engine-queue NoOps placed immediately before the instruction.
    """
    counter = 0
    for f in nc.m.functions:
        for blk in f.blocks:
            new = []
            for inst in blk.instructions:
                si = getattr(inst, "sync_info", None)
                if si is not None and len(si.on_wait) > 1:
                    waits = list(si.on_wait)
                    assert len(si.on_update) <= 1, inst
                    for wt in waits[:-1]:
                        counter += 1
                        new.append(mybir.InstNoOp(
                            name=f"I-waitnop-{counter}",
                            engine=inst.engine,
                            sync_info=mybir.SyncInfo(on_wait=[wt],
                                                     on_update=[]),
                        ))
                    inst.sync_info = mybir.SyncInfo(
                        on_wait=[waits[-1]], on_update=list(si.on_update))
                new.append(inst)
            blk.instructions = new
    return nc


def _make_in_maps(p, q, w):
    p = np.ascontiguousarray(p, dtype=np.float32)
    q = np.ascontiguousarray(q, dtype=np.float32)
    w = np.ascontiguousarray(w, dtype=np.float32)
    bf = ml_dtypes.bfloat16
    in_maps = []
    for c in range(NCORES):
        sl = slice(c * BP, (c + 1) * BP)
        ps = p[sl]
        qs = q[sl]
        # pnp[gi, r, jb, i, :] = p[b0+jb, i*100+r, :]
        pnp = np.ascontiguousarray(
            ps.reshape(NG, 4, 4, R, H).transpose(0, 3, 1, 2, 4).astype(bf))
        # ptp[gi, d, jb, kc, l] = p[b0+jb, l, kc*128+d]
        ptp = np.ascontiguousarray(
            ps.transpose(0, 2, 1).reshape(NG, 4, 2, 128, LP)
            .transpose(0, 3, 1, 2, 4))
        # qtp[d, kc, b*100+l] = q[b, l, kc*128+d]
        qtp = np.ascontiguousarray(
            qs.transpose(2, 0, 1).reshape(2, 128, BP * LQ).transpose(1, 0, 2))
        qnp = np.ascontiguousarray(qs.transpose(1, 0, 2).astype(bf))
        wtp = np.ascontiguousarray(w.reshape(2, 128, H).transpose(1, 0, 2))
        in_maps.append({"pnp": pnp, "ptp": ptp, "qtp": qtp, "qnp": qnp,
                        "wtp": wtp})
    return in_maps


def _assemble(res_c):
    """Rebuild (BP, LP, 4H) float32 from the permuted device outputs."""
    out = np.empty((BP, LP, 4 * H), np.float32)
    gP = np.asarray(res_c["gP"])      # (NG, R, 4, 4, H)
    gHt = np.asarray(res_c["gHt"])    # (NG, 4, H)
    gG2 = np.asarray(res_c["gG2"])    # (NG, R, 4, 4, 2H)
    out[:, :, 0:H] = (
        gP.transpose(0, 2, 3, 1, 4).reshape(BP, LP, H).astype(np.float32))
    out[:, :, H:2 * H] = np.broadcast_to(
        gHt.reshape(BP, 1, H).astype(np.float32), (BP, LP, H))
    out[:, :, 2 * H:4 * H] = (
        gG2.transpose(0, 2, 3, 1, 4).reshape(BP, LP, 2 * H)
        .astype(np.float32))
    return out


def run(p, q, w, trace=False):
    nc = legalize_waits(build_nc())
    res = run_bass_kernel_spmd(
        nc, _make_in_maps(p, q, w), list(range(NCORES)), trace=trace)
    out = np.concatenate(
        [_assemble(res.results[c]) for c in range(NCORES)], axis=0)
    return out, res


def kernel(p, q, w):
    out, _ = run(p, q, w, trace=False)
    return out


# revision 10
# speedup vs baseline: 1.5911x; 1.0280x over previous
"""BiDAF attention kernel for trn2 (8 NeuronCores, pure data parallel).

v4: mixed-precision, descriptor-minimized DMA.

- S = q W p chain stays fp32 (softmax logits have std ~256; bf16's ~1.5
  absolute logit error flips argmaxes and breaks both softmaxes).  Every
  other tensor is bf16.
- All DRAM tensors are host-side pre-permuted to exactly match the SBUF
  tile layouts, so each DMA moves per-partition-contiguous 8-16KB runs:
  ~100 descriptors per transfer instead of ~1600x512B.  (HWDGE
  descriptor generation costs ~2ns/descriptor of sequencer time and
  small descriptors also cap HBM throughput.)
- The C2Q softmax denominator rides the U matmul: per chunk, with the
  a-chunk weights already loaded, a second N=1 matmul against a ones
  column produces RS per LP row in a tiny PSUM tile (no activation
  accumulators).  Normalization is fused into the P*U elementwise op via
  scalar_tensor_tensor: (psU * rcp) * Pn.
- Ht rows are broadcast across partitions by a small SBUF->SBUF
  stride-0 DMA; the DRAM Ht block is written once (1, 4, 256) per group
  and the host tiles it to (400, 256) during unshard.
"""

from contextlib import ExitStack

import numpy as np
import ml_dtypes

import concourse.bass as bass
import concourse.mybir as mybir
import concourse.tile as tile
from concourse.bass_utils import run_bass_kernel_spmd
from concourse.masks import make_identity

F32 = mybir.dt.float32
BF16 = mybir.dt.bfloat16
AX = mybir.AxisListType
ALU = mybir.AluOpType
ACTF = mybir.ActivationFunctionType

B, LP, LQ, H = 128, 400, 100, 256
NCORES = 8
BP = B // NCORES   # batches per core
NG = BP // 4       # 4-batch groups per core
R = 100            # LP chunk rows (4 uniform chunks)


def build_nc():
    nc = bass.Bass("TRN2", target_bir_lowering=False, debug=False)

    # host-permuted layouts (match SBUF tiles exactly)
    pnp = nc.dram_tensor("pnp", [NG, R, 4, 4, H], BF16, kind="ExternalInput")
    ptp = nc.dram_tensor("ptp", [NG, 128, 4, 2, LP], F32,
                         kind="ExternalInput")
    qtp = nc.dram_tensor("qtp", [128, 2, BP * LQ], F32, kind="ExternalInput")
    qnp = nc.dram_tensor("qnp", [LQ, BP, H], BF16, kind="ExternalInput")
    wtp = nc.dram_tensor("wtp", [128, 2, H], F32, kind="ExternalInput")
    gP = nc.dram_tensor("gP", [NG, R, 4, 4, H], BF16, kind="ExternalOutput")
    gHt = nc.dram_tensor("gHt", [NG, 4, H], BF16, kind="ExternalOutput")
    gG2 = nc.dram_tensor("gG2", [NG, R, 4, 4, 2 * H], BF16,
                         kind="ExternalOutput")

    with tile.TileContext(nc) as tc, ExitStack() as ctx:
        cpool = ctx.enter_context(tc.tile_pool(name="consts", bufs=1))
        wp = ctx.enter_context(tc.tile_pool(name="work", bufs=2))
        pst = ctx.enter_context(tc.tile_pool(name="pst", bufs=2, space="PSUM"))
        pam = ctx.enter_context(tc.tile_pool(name="pam", bufs=2, space="PSUM"))
        ppu = ctx.enter_context(tc.tile_pool(name="ppu", bufs=2, space="PSUM"))
        ppt = ctx.enter_context(tc.tile_pool(name="ppt", bufs=2, space="PSUM"))

        # ---- constants ----
        identF = cpool.tile([128, 128], F32)
        make_identity(nc, identF[:])
        identB = cpool.tile([128, 128], BF16)
        make_identity(nc, identB[:])
        ones_rF = cpool.tile([1, 128], F32)
        nc.vector.memset(ones_rF[:], 1.0)
        onesCb = cpool.tile([128, 1], BF16)
        nc.vector.memset(onesCb[:], 1.0)
        ones_rB = cpool.tile([1, 128], BF16)
        nc.vector.memset(ones_rB[:], 1.0)

        Wt = cpool.tile([128, 2, H], F32)
        nc.sync.dma_start(Wt[:], wtp[:])
        QtA = cpool.tile([128, 2, BP * LQ], F32)
        AqA = cpool.tile([128, 2, BP * LQ], F32)
        QnA = cpool.tile([LQ, BP, H], BF16)

        def load_qt(gi):
            nc.sync.dma_start(QtA[:, :, gi * 400:(gi + 1) * 400],
                              qtp[:, :, gi * 400:(gi + 1) * 400])

        def compute_aq(gi):
            # Aq[d, (b l)] = sum_h w[h, d] q[b, l, h]  (fp32)
            for ms in range(2):
                psAq = pst.tile([128, 400], F32, tag="st")
                for kc in range(2):
                    nc.tensor.matmul(
                        psAq[:],
                        Wt[:, kc, ms * 128:(ms + 1) * 128],
                        QtA[:, kc, gi * 400:(gi + 1) * 400],
                        start=(kc == 0), stop=(kc == 1),
                    )
                nc.scalar.copy(AqA[:, ms, gi * 400:(gi + 1) * 400], psAq[:])

        load_qt(0)
        PtGs, PnGs = {}, {}

        def load_group(gi):
            PtG_t = wp.tile([128, 4, 2, LP], F32, tag="PtG", name=f"PtG{gi}")
            nc.sync.dma_start(PtG_t[:], ptp[gi])
            PtGs[gi] = PtG_t
            PnG_t = wp.tile([R, 4, 4, H], BF16, tag="PnG", bufs=3,
                            name=f"PnG{gi}")
            nc.sync.dma_start(PnG_t[:], pnp[gi])
            PnGs[gi] = PnG_t

        load_group(0)
        compute_aq(0)
        for gi in range(1, NG):
            load_qt(gi)
        nc.scalar.dma_start(QnA[:], qnp[:])

        for gi in range(NG):
            b0 = gi * 4
            # ---------------- group loads (prefetched) ----------------
            PtG = PtGs.pop(gi)
            PnG = PnGs.pop(gi)
            if gi + 1 < NG:
                compute_aq(gi + 1)
                load_group(gi + 1)

            GtG = wp.tile([R, 4, 4, 2 * H], BF16, tag="GtG")
            HtG = wp.tile([R, 4, H], BF16, tag="HtG")

            grp = []
            NMN4 = wp.tile([R, 4], F32, tag="NMN4")
            # ---------------- phase A: C2Q per batch ----------------
            for j in range(4):
                b = b0 + j
                bq = b * LQ
                # S^T = p @ Aq  (chunked LP x LQ), fp32
                psSt = pst.tile([R, 4, LQ], F32, tag="st")
                for i in range(4):
                    for kc in range(2):
                        nc.tensor.matmul(
                            psSt[:, i, :],
                            PtG[:, j, kc, i * R:(i + 1) * R],
                            AqA[:, kc, bq:bq + LQ],
                            start=(kc == 0), stop=(kc == 1),
                        )
                NM = wp.tile([R, 4], F32, tag="NM", bufs=6)
                nc.vector.tensor_reduce(NM[:], psSt[:], axis=AX.X,
                                        op=ALU.max, negate=True)
                E = wp.tile([R, 4, LQ], BF16, tag="E")
                for i in range(4):
                    nc.scalar.activation(
                        E[:, i, :], psSt[:, i, :], ACTF.Exp,
                        bias=NM[:, i:i + 1],
                    )

                # a = E^T (unnormalized); U^T = a_chunk @ Qn, RS rides along
                psAm = pam.tile([R, 4, LQ], BF16, tag="am")
                for i in range(4):
                    nc.tensor.transpose(
                        psAm[0:LQ, i, 0:R], E[:, i, :], identB[0:R, 0:R])
                Am = wp.tile([LQ, 4, R], BF16, tag="Am")
                nc.scalar.copy(Am[:], psAm[0:LQ, :, :])
                RS = wp.tile([R, 4], F32, tag="RS")
                nc.vector.tensor_reduce(RS[:], E[:], axis=AX.X, op=ALU.add)
                RCP = wp.tile([R, 4], F32, tag="RCP")
                nc.vector.reciprocal(RCP[:], RS[:])
                psUs = []
                for half in range(2):
                    psU = ppu.tile([R, 2, H], F32, tag="u")
                    psUs.append(psU)
                    for ih in range(2):
                        i = half * 2 + ih
                        nc.tensor.matmul(
                            psU[:, ih, :], Am[:, i, :], QnA[:, b, :],
                            start=True, stop=True,
                        )
                for half in range(2):
                    for ih in range(2):
                        i = half * 2 + ih
                        nc.vector.scalar_tensor_tensor(
                            GtG[:, j, i, 0:H],
                            psUs[half][:, ih, :], RCP[:, i:i + 1],
                            PnG[:, j, i, :],
                            op0=ALU.mult, op1=ALU.mult,
                        )
                nc.vector.tensor_reduce(NMN4[:, j:j + 1], NM[:], axis=AX.X,
                                        op=ALU.min)
                grp.append((b, NM))

            # ---------------- group Q2C glue ----------------
            psTB = ppt.tile([128, 260], F32, tag="tiny")
            nc.tensor.transpose(psTB[0:4, 0:R], NMN4[:], identF[0:R, 0:R])
            ngmin4 = wp.tile([4, 1], F32, tag="ngmin4")   # -gmax per batch
            nc.vector.tensor_reduce(ngmin4[:], psTB[0:4, 0:R], axis=AX.X,
                                    op=ALU.min)
            psT2 = ppt.tile([128, 260], F32, tag="tiny")
            nc.tensor.transpose(psT2[0:1, 0:4], ngmin4[:], identF[0:4, 0:4])
            ngr = wp.tile([1, 4], F32, tag="ngr")
            nc.vector.tensor_copy(ngr[:], psT2[0:1, 0:4])
            psB4 = ppt.tile([128, 260], F32, tag="tiny")
            nc.tensor.matmul(psB4[0:R, 0:4], ones_rF[0:1, 0:R], ngr[:],
                             start=True, stop=True)
            nb4 = wp.tile([R, 4], F32, tag="nb4")
            nc.vector.tensor_copy(nb4[:], psB4[0:R, 0:4])

            # ---------------- phase B: Q2C per batch ----------------
            for j in range(4):
                b, NM = grp[j]
                EQ = wp.tile([R, 4], BF16, tag="EQ")
                nc.scalar.activation(EQ[:], NM[:], ACTF.Exp,
                                     bias=nb4[:, j:j + 1], scale=-1.0)
                psHr = ppt.tile([128, 260], F32, tag="tiny")
                for i in range(4):
                    nc.tensor.matmul(
                        psHr[0:1, 0:H],
                        EQ[:, i:i + 1],
                        PnG[:, j, i, :],
                        start=(i == 0), stop=(i == 3),
                    )
                nc.tensor.matmul(psHr[0:1, H:H + 4], onesCb[0:R, :], EQ[:],
                                 start=True, stop=True)
                smv = wp.tile([1, 1], F32, tag="smv")
                nc.vector.tensor_reduce(smv[:], psHr[0:1, H:H + 4],
                                        axis=AX.X, op=ALU.add)
                rq = wp.tile([1, 1], F32, tag="rq")
                nc.vector.reciprocal(rq[:], smv[:])
                hrow = wp.tile([1, H], BF16, tag="hrow")
                nc.scalar.mul(hrow[:], psHr[0:1, 0:H], rq[:])
                # broadcast h row across partitions via ones x hrow matmul
                psHt = ppt.tile([128, 260], F32, tag="tiny")
                nc.tensor.matmul(psHt[0:R, 0:H], ones_rB[0:1, 0:R],
                                 hrow[:], start=True, stop=True)
                nc.scalar.copy(HtG[:, j, :], psHt[0:R, 0:H])
                nc.gpsimd.tensor_tensor(
                    GtG[:, j, 0:2, H:2 * H], PnG[:, j, 0:2, :],
                    HtG[:, j:j + 1, :].broadcast_to((R, 2, H)), op=ALU.mult)
                nc.vector.tensor_tensor(
                    GtG[:, j, 2:4, H:2 * H], PnG[:, j, 2:4, :],
                    HtG[:, j:j + 1, :].broadcast_to((R, 2, H)), op=ALU.mult)

            # ---------------- group stores ----------------
            nc.scalar.dma_start(gP[gi], PnG[:])
            nc.scalar.dma_start(gHt[gi], HtG[0:1, :, :])
            nc.sync.dma_start(gG2[gi], GtG[:])

    return nc


def legalize_waits(nc):
    """Split multi-wait instructions into single-wait NoOps + instruction.

    The TPB ISA has exactly one (wait, update) EVENTS slot per 64B
    instruction; this walrus build refuses instructions with more than one
    sync wait ("Too many sync wait commands").  Tile's scheduler emits
    vector-clock waits freely, so legalize here: excess waits move onto
    engine-queue NoOps placed immediately before the instruction.
    """
    counter = 0
    for f in nc.m.functions:
        for blk in f.blocks:
            new = []
            for inst in blk.instructions:
                si = getattr(inst, "sync_info", None)
                if si is not None and len(si.on_wait) > 1:
                    waits = list(si.on_wait)
                    assert len(si.on_update) <= 1, inst
                    for wt in waits[:-1]:
                        counter += 1
                        new.append(mybir.InstNoOp(
                            name=f"I-waitnop-{counter}",
                            engine=inst.engine,
                            sync_info=mybir.SyncInfo(on_wait=[wt],
                                                     on_update=[]),
                        ))
                    inst.sync_info = mybir.SyncInfo(
                        on_wait=[waits[-1]], on_update=list(si.on_update))
                new.append(inst)
            blk.instructions = new
    return nc


def _make_in_maps(p, q, w):
    p = np.ascontiguousarray(p, dtype=np.float32)
    q = np.ascontiguousarray(q, dtype=np.float32)
    w = np.ascontiguousarray(w, dtype=np.float32)
    bf = ml_dtypes.bfloat16
    in_maps = []
    for c in range(NCORES):
        sl = slice(c * BP, (c + 1) * BP)
        ps = p[sl]
        qs = q[sl]
        # pnp[gi, r, jb, i, :] = p[b0+jb, i*100+r, :]
        pnp = np.ascontiguousarray(
            ps.reshape(NG, 4, 4, R, H).transpose(0, 3, 1, 2, 4).astype(bf))
        # ptp[gi, d, jb, kc, l] = p[b0+jb, l, kc*128+d]
        ptp = np.ascontiguousarray(
            ps.transpose(0, 2, 1).reshape(NG, 4, 2, 128, LP)
            .transpose(0, 3, 1, 2, 4))
        # qtp[d, kc, b*100+l] = q[b, l, kc*128+d]
        qtp = np.ascontiguousarray(
            qs.transpose(2, 0, 1).reshape(2, 128, BP * LQ).transpose(1, 0, 2))
        qnp = np.ascontiguousarray(qs.transpose(1, 0, 2).astype(bf))
        wtp = np.ascontiguousarray(w.reshape(2, 128, H).transpose(1, 0, 2))
        in_maps.append({"pnp": pnp, "ptp": ptp, "qtp": qtp, "qnp": qnp,
                        "wtp": wtp})
    return in_maps


def _assemble(res_c):
    """Rebuild (BP, LP, 4H) float32 from the permuted device outputs."""
    out = np.empty((BP, LP, 4 * H), np.float32)
    gP = np.asarray(res_c["gP"])      # (NG, R, 4, 4, H)
    gHt = np.asarray(res_c["gHt"])    # (NG, 4, H)
    gG2 = np.asarray(res_c["gG2"])    # (NG, R, 4, 4, 2H)
    out[:, :, 0:H] = (
        gP.transpose(0, 2, 3, 1, 4).reshape(BP, LP, H).astype(np.float32))
    out[:, :, H:2 * H] = np.broadcast_to(
        gHt.reshape(BP, 1, H).astype(np.float32), (BP, LP, H))
    out[:, :, 2 * H:4 * H] = (
        gG2.transpose(0, 2, 3, 1, 4).reshape(BP, LP, 2 * H)
        .astype(np.float32))
    return out


def run(p, q, w, trace=False):
    nc = legalize_waits(build_nc())
    res = run_bass_kernel_spmd(
        nc, _make_in_maps(p, q, w), list(range(NCORES)), trace=trace)
    out = np.concatenate(
        [_assemble(res.results[c]) for c in range(NCORES)], axis=0)
    return out, res


def kernel(p, q, w):
    out, _ = run(p, q, w, trace=False)
    return out


# revision 11
# speedup vs baseline: 1.6079x; 1.0105x over previous
"""BiDAF attention kernel for trn2 (8 NeuronCores, pure data parallel).

v4: mixed-precision, descriptor-minimized DMA.

- S = q W p chain stays fp32 (softmax logits have std ~256; bf16's ~1.5
  absolute logit error flips argmaxes and breaks both softmaxes).  Every
  other tensor is bf16.
- All DRAM tensors are host-side pre-permuted to exactly match the SBUF
  tile layouts, so each DMA moves per-partition-contiguous 8-16KB runs:
  ~100 descriptors per transfer instead of ~1600x512B.  (HWDGE
  descriptor generation costs ~2ns/descriptor of sequencer time and
  small descriptors also cap HBM throughput.)
- The C2Q softmax denominator rides the U matmul: per chunk, with the
  a-chunk weights already loaded, a second N=1 matmul against a ones
  column produces RS per LP row in a tiny PSUM tile (no activation
  accumulators).  Normalization is fused into the P*U elementwise op via
  scalar_tensor_tensor: (psU * rcp) * Pn.
- Ht rows are broadcast across partitions by a small SBUF->SBUF
  stride-0 DMA; the DRAM Ht block is written once (1, 4, 256) per group
  and the host tiles it to (400, 256) during unshard.
"""

from contextlib import ExitStack

import numpy as np
import ml_dtypes

import concourse.bass as bass
import concourse.mybir as mybir
import concourse.tile as tile
from concourse.bass_utils import run_bass_kernel_spmd
from concourse.masks import make_identity

F32 = mybir.dt.float32
BF16 = mybir.dt.bfloat16
AX = mybir.AxisListType
ALU = mybir.AluOpType
ACTF = mybir.ActivationFunctionType

B, LP, LQ, H = 128, 400, 100, 256
NCORES = 8
BP = B // NCORES   # batches per core
NG = BP // 4       # 4-batch groups per core
R = 100            # LP chunk rows (4 uniform chunks)


def build_nc():
    nc = bass.Bass("TRN2", target_bir_lowering=False, debug=False)

    # host-permuted layouts (match SBUF tiles exactly)
    pnp = nc.dram_tensor("pnp", [NG, R, 4, 4, H], BF16, kind="ExternalInput")
    ptp = nc.dram_tensor("ptp", [NG, 128, 4, 2, LP], F32,
                         kind="ExternalInput")
    qtp = nc.dram_tensor("qtp", [128, 2, BP * LQ], F32, kind="ExternalInput")
    qnp = nc.dram_tensor("qnp", [LQ, BP, H], BF16, kind="ExternalInput")
    wtp = nc.dram_tensor("wtp", [128, 2, H], F32, kind="ExternalInput")
    gP = nc.dram_tensor("gP", [NG, R, 4, 4, H], BF16, kind="ExternalOutput")
    gHt = nc.dram_tensor("gHt", [NG, 4, H], BF16, kind="ExternalOutput")
    gG2 = nc.dram_tensor("gG2", [NG, R, 4, 4, 2 * H], BF16,
                         kind="ExternalOutput")

    with tile.TileContext(nc) as tc, ExitStack() as ctx:
        cpool = ctx.enter_context(tc.tile_pool(name="consts", bufs=1))
        wp = ctx.enter_context(tc.tile_pool(name="work", bufs=2))
        pst = ctx.enter_context(tc.tile_pool(name="pst", bufs=2, space="PSUM"))
        pam = ctx.enter_context(tc.tile_pool(name="pam", bufs=2, space="PSUM"))
        ppu = ctx.enter_context(tc.tile_pool(name="ppu", bufs=2, space="PSUM"))
        ppt = ctx.enter_context(tc.tile_pool(name="ppt", bufs=2, space="PSUM"))

        # ---- constants ----
        identF = cpool.tile([128, 128], F32)
        make_identity(nc, identF[:])
        identB = cpool.tile([128, 128], BF16)
        make_identity(nc, identB[:])
        ones_rF = cpool.tile([1, 128], F32)
        nc.vector.memset(ones_rF[:], 1.0)
        onesCb = cpool.tile([128, 1], BF16)
        nc.vector.memset(onesCb[:], 1.0)
        ones_rB = cpool.tile([1, 128], BF16)
        nc.vector.memset(ones_rB[:], 1.0)

        Wt = cpool.tile([128, 2, H], F32)
        nc.sync.dma_start(Wt[:], wtp[:])
        QtA = cpool.tile([128, 2, BP * LQ], F32)
        AqA = cpool.tile([128, 2, BP * LQ], F32)
        QnA = cpool.tile([LQ, BP, H], BF16)

        def load_qt(gi):
            nc.sync.dma_start(QtA[:, :, gi * 400:(gi + 1) * 400],
                              qtp[:, :, gi * 400:(gi + 1) * 400])

        def compute_aq(gi):
            # Aq[d, (b l)] = sum_h w[h, d] q[b, l, h]  (fp32)
            for ms in range(2):
                psAq = pst.tile([128, 400], F32, tag="st")
                for kc in range(2):
                    nc.tensor.matmul(
                        psAq[:],
                        Wt[:, kc, ms * 128:(ms + 1) * 128],
                        QtA[:, kc, gi * 400:(gi + 1) * 400],
                        start=(kc == 0), stop=(kc == 1),
                    )
                nc.scalar.copy(AqA[:, ms, gi * 400:(gi + 1) * 400], psAq[:])

        # keep the PE busy (HAM-warm) while the first inputs stream in
        psW = pst.tile([128, 128], BF16, tag="st")
        for _ in range(48):
            nc.tensor.transpose(psW[:], identB[:], identB[:])

        load_qt(0)
        PtGs, PnGs = {}, {}

        def load_group(gi):
            PtG_t = wp.tile([128, 4, 2, LP], F32, tag="PtG", name=f"PtG{gi}")
            nc.sync.dma_start(PtG_t[:], ptp[gi])
            PtGs[gi] = PtG_t
            PnG_t = wp.tile([R, 4, 4, H], BF16, tag="PnG", bufs=3,
                            name=f"PnG{gi}")
            nc.sync.dma_start(PnG_t[:], pnp[gi])
            PnGs[gi] = PnG_t

        load_group(0)
        compute_aq(0)
        for gi in range(1, NG):
            load_qt(gi)
        nc.scalar.dma_start(QnA[:], qnp[:])

        for gi in range(NG):
            b0 = gi * 4
            # ---------------- group loads (prefetched) ----------------
            PtG = PtGs.pop(gi)
            PnG = PnGs.pop(gi)
            if gi + 1 < NG:
                compute_aq(gi + 1)
                load_group(gi + 1)

            GtG = wp.tile([R, 4, 4, 2 * H], BF16, tag="GtG")
            HtG = wp.tile([R, 4, H], BF16, tag="HtG")
            # gP only needs the loaded PnG; issue its store up front
            nc.scalar.dma_start(gP[gi], PnG[:])

            for j in range(4):
                b = b0 + j
                bq = b * LQ
                # S^T = p @ Aq  (chunked LP x LQ), fp32
                psSt = pst.tile([R, 4, LQ], F32, tag="st")
                for i in range(4):
                    for kc in range(2):
                        nc.tensor.matmul(
                            psSt[:, i, :],
                            PtG[:, j, kc, i * R:(i + 1) * R],
                            AqA[:, kc, bq:bq + LQ],
                            start=(kc == 0), stop=(kc == 1),
                        )
                NM = wp.tile([R, 4], F32, tag="NM", bufs=6)
                nc.vector.tensor_reduce(NM[:], psSt[:], axis=AX.X,
                                        op=ALU.max, negate=True)
                E = wp.tile([R, 4, LQ], BF16, tag="E")
                for i in range(4):
                    nc.scalar.activation(
                        E[:, i, :], psSt[:, i, :], ACTF.Exp,
                        bias=NM[:, i:i + 1],
                    )
                # per-batch Q2C stabilizer: gmax_b = -min over all chunks
                NMN = wp.tile([R, 1], F32, tag="NMN")
                nc.vector.tensor_reduce(NMN[:], NM[:], axis=AX.X, op=ALU.min)
                psT = ppt.tile([128, 260], F32, tag="tiny")
                nc.tensor.transpose(psT[0:1, 0:R], NMN[:], identF[0:R, 0:R])
                gneg = wp.tile([1, 1], F32, tag="gneg")
                nc.vector.tensor_reduce(gneg[:], psT[0:1, 0:R], axis=AX.X,
                                        op=ALU.min)
                psB = ppt.tile([128, 260], F32, tag="tiny")
                nc.tensor.matmul(psB[0:R, 0:1], ones_rF[0:1, 0:R], gneg[:],
                                 start=True, stop=True)
                nbv = wp.tile([R, 1], F32, tag="nbv")
                nc.vector.tensor_copy(nbv[:], psB[0:R, 0:1])

                # a = E^T (unnormalized); U^T = a_chunk @ Qn, RS rides along
                psAm = pam.tile([R, 4, LQ], BF16, tag="am")
                for i in range(4):
                    nc.tensor.transpose(
                        psAm[0:LQ, i, 0:R], E[:, i, :], identB[0:R, 0:R])
                Am = wp.tile([LQ, 4, R], BF16, tag="Am")
                nc.scalar.copy(Am[:], psAm[0:LQ, :, :])
                RS = wp.tile([R, 4], F32, tag="RS")
                nc.vector.tensor_reduce(RS[:], E[:], axis=AX.X, op=ALU.add)
                RCP = wp.tile([R, 4], F32, tag="RCP")
                nc.vector.reciprocal(RCP[:], RS[:])
                psUs = []
                for half in range(2):
                    psU = ppu.tile([R, 2, H], F32, tag="u")
                    psUs.append(psU)
                    for ih in range(2):
                        i = half * 2 + ih
                        nc.tensor.matmul(
                            psU[:, ih, :], Am[:, i, :], QnA[:, b, :],
                            start=True, stop=True,
                        )
                for half in range(2):
                    for ih in range(2):
                        i = half * 2 + ih
                        nc.vector.scalar_tensor_tensor(
                            GtG[:, j, i, 0:H],
                            psUs[half][:, ih, :], RCP[:, i:i + 1],
                            PnG[:, j, i, :],
                            op0=ALU.mult, op1=ALU.mult,
                        )
                # ---------------- Q2C for this batch ----------------
                EQ = wp.tile([R, 4], BF16, tag="EQ")
                nc.scalar.activation(EQ[:], NM[:], ACTF.Exp,
                                     bias=nbv[:, 0:1], scale=-1.0)
                psHr = ppt.tile([128, 260], F32, tag="tiny")
                for i in range(4):
                    nc.tensor.matmul(
                        psHr[0:1, 0:H],
                        EQ[:, i:i + 1],
                        PnG[:, j, i, :],
                        start=(i == 0), stop=(i == 3),
                    )
                nc.tensor.matmul(psHr[0:1, H:H + 4], onesCb[0:R, :], EQ[:],
                                 start=True, stop=True)
                smv = wp.tile([1, 1], F32, tag="smv")
                nc.vector.tensor_reduce(smv[:], psHr[0:1, H:H + 4],
                                        axis=AX.X, op=ALU.add)
                rq = wp.tile([1, 1], F32, tag="rq")
                nc.vector.reciprocal(rq[:], smv[:])
                hrow = wp.tile([1, H], BF16, tag="hrow")
                nc.scalar.mul(hrow[:], psHr[0:1, 0:H], rq[:])
                # broadcast h row across partitions via ones x hrow matmul
                psHt = ppt.tile([128, 260], F32, tag="tiny")
                nc.tensor.matmul(psHt[0:R, 0:H], ones_rB[0:1, 0:R],
                                 hrow[:], start=True, stop=True)
                nc.scalar.copy(HtG[:, j, :], psHt[0:R, 0:H])
                nc.gpsimd.tensor_tensor(
                    GtG[:, j, 0:3, H:2 * H], PnG[:, j, 0:3, :],
                    HtG[:, j:j + 1, :].broadcast_to((R, 3, H)), op=ALU.mult)
                nc.vector.tensor_tensor(
                    GtG[:, j, 3:4, H:2 * H], PnG[:, j, 3:4, :],
                    HtG[:, j:j + 1, :].broadcast_to((R, 1, H)), op=ALU.mult)

            # ---------------- group stores ----------------
            nc.scalar.dma_start(gHt[gi], HtG[0:1, :, :])
            nc.sync.dma_start(gG2[gi], GtG[:])

    return nc


def legalize_waits(nc):
    """Split multi-wait instructions into single-wait NoOps + instruction.

    The TPB ISA has exactly one (wait, update) EVENTS slot per 64B
    instruction; this walrus build refuses instructions with more than one
    sync wait ("Too many sync wait commands").  Tile's scheduler emits
    vector-clock waits freely, so legalize here: excess waits move onto
    engine-queue NoOps placed immediately before the instruction.
    """
    counter = 0
    for f in nc.m.functions:
        for blk in f.blocks:
            new = []
            for inst in blk.instructions:
                si = getattr(inst, "sync_info", None)
                if si is not None and len(si.on_wait) > 1:
                    waits = list(si.on_wait)
                    assert len(si.on_update) <= 1, inst
                    for wt in waits[:-1]:
                        counter += 1
                        new.append(mybir.InstNoOp(
                            name=f"I-waitnop-{counter}",
                            engine=inst.engine,
                            sync_info=mybir.SyncInfo(on_wait=[wt],
                                                     on_update=[]),
                        ))
                    inst.sync_info = mybir.SyncInfo(
                        on_wait=[waits[-1]], on_update=list(si.on_update))
                new.append(inst)
            blk.instructions = new
    return nc


def _make_in_maps(p, q, w):
    p = np.ascontiguousarray(p, dtype=np.float32)
    q = np.ascontiguousarray(q, dtype=np.float32)
    w = np.ascontiguousarray(w, dtype=np.float32)
    bf = ml_dtypes.bfloat16
    in_maps = []
    for c in range(NCORES):
        sl = slice(c * BP, (c + 1) * BP)
        ps = p[sl]
        qs = q[sl]
        # pnp[gi, r, jb, i, :] = p[b0+jb, i*100+r, :]
        pnp = np.ascontiguousarray(
            ps.reshape(NG, 4, 4, R, H).transpose(0, 3, 1, 2, 4).astype(bf))
        # ptp[gi, d, jb, kc, l] = p[b0+jb, l, kc*128+d]
        ptp = np.ascontiguousarray(
            ps.transpose(0, 2, 1).reshape(NG, 4, 2, 128, LP)
            .transpose(0, 3, 1, 2, 4))
        # qtp[d, kc, b*100+l] = q[b, l, kc*128+d]
        qtp = np.ascontiguousarray(
            qs.transpose(2, 0, 1).reshape(2, 128, BP * LQ).transpose(1, 0, 2))
        qnp = np.ascontiguousarray(qs.transpose(1, 0, 2).astype(bf))
        wtp = np.ascontiguousarray(w.reshape(2, 128, H).transpose(1, 0, 2))
        in_maps.append({"pnp": pnp, "ptp": ptp, "qtp": qtp, "qnp": qnp,
                        "wtp": wtp})
    return in_maps


def _assemble(res_c):
    """Rebuild (BP, LP, 4H) float32 from the permuted device outputs."""
    out = np.empty((BP, LP, 4 * H), np.float32)
    gP = np.asarray(res_c["gP"])      # (NG, R, 4, 4, H)
    gHt = np.asarray(res_c["gHt"])    # (NG, 4, H)
    gG2 = np.asarray(res_c["gG2"])    # (NG, R, 4, 4, 2H)
    out[:, :, 0:H] = (
        gP.transpose(0, 2, 3, 1, 4).reshape(BP, LP, H).astype(np.float32))
    out[:, :, H:2 * H] = np.broadcast_to(
        gHt.reshape(BP, 1, H).astype(np.float32), (BP, LP, H))
    out[:, :, 2 * H:4 * H] = (
        gG2.transpose(0, 2, 3, 1, 4).reshape(BP, LP, 2 * H)
        .astype(np.float32))
    return out


def run(p, q, w, trace=False):
    nc = legalize_waits(build_nc())
    res = run_bass_kernel_spmd(
        nc, _make_in_maps(p, q, w), list(range(NCORES)), trace=trace)
    out = np.concatenate(
        [_assemble(res.results[c]) for c in range(NCORES)], axis=0)
    return out, res


def kernel(p, q, w):
    out, _ = run(p, q, w, trace=False)
    return out
